# revision 49
# baseline (speedup 1.0000x reference)
"""Sequence-parallel dense attention kernel for 8 Trainium2 NeuronCores.

Math (reference):
    h = x @ W1.T + b1                  [N, H]
    q/k/v = h @ W{q,k,v}.T + b{q,k,v}  [N, H]
    A = softmax(q @ k.T / sqrt(H))     [N, N]
    out = (h + A @ v) @ W2.T + b2      [N]

Restructuring (v3, fp8 + asymmetric pipelined gather):
  * All projections fold through lin1 on the host so q, k, z, resid all come
    straight from x:  q = x@(Wq@W1).T + bq', k = x@(Wk@W1).T + bk',
    z = v@w2 = x@zw + zc0, resid = h@w2 = x@wr + cr.  No h is ever computed.
  * Softmax rows sum to one, so out[n] = resid[n] + (A_un[n,:]@z)/(A_un[n,:]@1)
    with A_un = exp(scores); W2 is applied to V before attention.
  * q.T/k.T are scaled by 64 and stored fp8(e4m3); score matmuls use
    perf_mode=DoubleRow (contract 256 in one pass).  exp(score) in [0.28, 3.6]
    is stored fp8 and the num/den reduction is also a DoubleRow matmul
    (two 128-row nk chunks per pass) against a [z-replicated | ones]
    stationary (out rows 0..63 = num, 64..127 = den), built for all 32 pairs
    with a few broadcast DVE copies.
  * exp is split across engines: ScalarE ACT exp for ~55% of tiles, VectorE
    computes the rest as int8(a*score + b) bitcast to fp8 (Schraudolph: the
    int8 bit pattern IS an fp8 exp approximation).
  * The k.T+z gather is fp8 and split asymmetrically (cols 0:256, 256:1024):
    each AllGather costs ~12us fixed + ~bandwidth, so the first split is
    small to arm early (~17us) and expose the minimum, while the large
    remainder gathers behind the first split's 8 score pairs.  DMAs are
    batched (one dma_start per logical block) because each dma_start costs
    ~650ns of serial Sync-engine issue time.

Sharding: rows of x across 8 cores (S = N/8 per core); each core computes its
S x N score block transposed (nk on partitions) and reduces it on the PE.
"""

import numpy as np

N, D, H = 8192, 1024, 256
NC = 8
S = N // NC          # rows per core
NKC = N // 128       # 64 global nk chunks
NPAIR = NKC // 2     # 32 DoubleRow chunk pairs
# gather column splits: small first (arms early), big second (hidden)
SPLITS = [(0, 256), (256, 1024)]
CQS = [(e - s) // 128 for s, e in SPLITS]   # chunks per rank per split: [2, 6]
SCQ = 64.0           # fp8 q/k prescale
SC_EXP = 0.0625 / (SCQ * SCQ)   # exp( st * SC_EXP )
# Schraudolph fp8e4m3: i8 = rint(st*A8 + B8); bitcast int8 -> fp8 approximates
# exp(st * SC_EXP).  B8 = 56 - c_opt, c_opt = 0.30 (numerically tuned).
A8 = 8.0 * 1.4426950408889634 * SC_EXP
B8 = 56.0 - 0.30
LAG = 6              # reduce of pair i is emitted with score of pair i+LAG

_cache = {}


def _build_program():
    import concourse.tile as tile
    from concourse import bacc, mybir
    from concourse.masks import make_identity

    f32 = mybir.dt.float32
    f32r = mybir.dt.float32r
    fp8 = mybir.dt.float8e4
    i8 = mybir.dt.int8
    Ident = mybir.ActivationFunctionType.Identity
    Exp = mybir.ActivationFunctionType.Exp
    DR = mybir.MatmulPerfMode.DoubleRow
    Mul = mybir.AluOpType.mult
    Add = mybir.AluOpType.add

    nc = bacc.Bacc("TRN2", target_bir_lowering=False, debug=False, num_devices=NC)

    xT = nc.dram_tensor("xT", [D, S], f32r, kind="ExternalInput").ap()
    wk1T = nc.dram_tensor("wk1T", [D, H], f32r, kind="ExternalInput").ap()
    wq1T = nc.dram_tensor("wq1T", [D, H], f32r, kind="ExternalInput").ap()
    # packed small constants (per-partition columns):
    #   0-1 bk1*64 | 2-3 bq1*64 | col4: p0 zc0, p1 cr
    cpk = nc.dram_tensor("cpk", [128, 16], f32, kind="ExternalInput").ap()
    # zwp: col 2*dc = zw chunk dc (z row), col 2*dc+1 = wr chunk dc (resid row)
    zwp = nc.dram_tensor("zwp", [128, 16], f32r, kind="ExternalInput").ap()
    out_d = nc.dram_tensor("out", [1, S], f32, kind="ExternalOutput").ap()

    # split collectives: kt rows 0:256 ([hc*128+p, n]) + z row 256, fp8
    cc_in = [
        nc.dram_tensor(f"cc_in{t}", [H + 1, e - s], fp8).ap()
        for t, (s, e) in enumerate(SPLITS)
    ]
    cc_out = [
        nc.dram_tensor(
            f"cc_out{t}", [(H + 1) * NC, e - s], fp8, addr_space="Shared"
        ).ap()
        for t, (s, e) in enumerate(SPLITS)
    ]


    with tile.TileContext(nc) as tc:
        with (
            tc.tile_pool(name="consts", bufs=1) as consts,
            tc.tile_pool(name="xpool", bufs=8) as xpool,
            tc.tile_pool(name="work", bufs=1) as work,
            tc.tile_pool(name="expp", bufs=8) as expp,
            tc.tile_pool(name="stp", bufs=3, space="PSUM") as stp,
            tc.tile_pool(name="redp", bufs=1, space="PSUM") as redp,
        ):
            # ---- loads: x split-0 columns first so the first collective can
            # arm before the rest of x lands ----
            wk1sb = consts.tile([128, 8, H], f32r)
            wq1sb = consts.tile([128, 8, H], f32r)
            xts = [xpool.tile([128, S], f32r, tag="xt", name="xt") for _ in range(8)]
            nc.sync.dma_start(out=wk1sb, in_=wk1T.rearrange("(c p) h -> p c h", p=128))
            for dc in range(8):
                nc.sync.dma_start(
                    out=xts[dc][:, 0:256], in_=xT[dc * 128:(dc + 1) * 128, 0:256]
                )
            cpack = consts.tile([128, 16], f32)
            nc.sync.dma_start(out=cpack, in_=cpk)
            zwsb = consts.tile([128, 16], f32r)
            nc.sync.dma_start(out=zwsb, in_=zwp)
            for dc in range(8):
                nc.sync.dma_start(
                    out=xts[dc][:, 256:S], in_=xT[dc * 128:(dc + 1) * 128, 256:S]
                )
            nc.sync.dma_start(out=wq1sb, in_=wq1T.rearrange("(c p) h -> p c h", p=128))
            # warm the ACT exp table set before any real activation needs it
            dumm = consts.tile([1, 1], f32)
            nc.vector.memset(dumm, 0.0)
            dumo = consts.tile([1, 1], f32)
            nc.scalar.activation(out=dumo, in_=dumm, func=Exp)

            # ---- ktloc = Wk1 @ x.T (fp8, *64) and z/resid rows, by split;
            # each split ships to its collective as soon as it is done ----
            ktl8 = work.tile([128, 2, S], fp8)
            zr2f8 = work.tile([2, S], fp8)
            residsb2 = consts.tile([2, S], f32)
            for t, (cs, ce) in enumerate(SPLITS):
                # psum rounds of <=512 columns within the split
                subs = [(a, min(a + 512, ce)) for a in range(cs, ce, 512)]
                for (sa, sb) in subs:
                    w = sb - sa
                    for hc in range(2):
                        ps = stp.tile([128, w], f32, tag="st", name="ps")
                        for dc in range(8):
                            nc.tensor.matmul(
                                ps,
                                lhsT=wk1sb[:, dc, hc * 128:(hc + 1) * 128],
                                rhs=xts[dc][:, sa:sb],
                                start=(dc == 0),
                                stop=(dc == 7),
                            )
                        nc.scalar.activation(
                            out=ktl8[:, hc, sa:sb], in_=ps,
                            func=Ident, bias=cpack[:, hc:hc + 1], scale=SCQ,
                        )
                    psz = stp.tile([2, w], f32, tag="st", name="psz")
                    for dc in range(8):
                        nc.tensor.matmul(
                            psz,
                            lhsT=zwsb[:, 2 * dc:2 * dc + 2],
                            rhs=xts[dc][:, sa:sb],
                            start=(dc == 0),
                            stop=(dc == 7),
                        )
                    # rows [z, resid] + per-partition bias [zc0, cr]; the z
                    # row is used in fp8 (row 1 is an unused resid copy)
                    nc.vector.tensor_scalar_add(
                        zr2f8[:, sa:sb], psz[0:2, :], cpack[0:2, 4:5]
                    )
                    nc.vector.tensor_scalar_add(
                        residsb2[:, sa:sb], psz[0:2, :], cpack[0:2, 4:5]
                    )
                # ship this split to its collective input and gather it
                nc.sync.dma_start(
                    out=cc_in[t][0:H, :].rearrange("(i p) c -> p i c", p=128),
                    in_=ktl8[:, :, cs:ce],
                )
                nc.sync.dma_start(out=cc_in[t][H:H + 1, :], in_=zr2f8[0:1, cs:ce])
                nc.gpsimd.collective_compute(
                    "AllGather",
                    mybir.AluOpType.bypass,
                    replica_groups=[list(range(NC))],
                    ins=[cc_in[t][:]],
                    outs=[cc_out[t][:]],
                )

            # resid to partition 0 for the epilogue
            residsb = consts.tile([1, S], f32)
            nc.sync.dma_start(out=residsb, in_=residsb2[1:2, :])

            # ---- qT (overlaps the collectives) ----
            qt8 = work.tile([128, 2, S], fp8)
            for hc in range(2):
                for nt in range(2):
                    ps = stp.tile([128, 512], f32, tag="st", name="ps")
                    for dc in range(8):
                        nc.tensor.matmul(
                            ps,
                            lhsT=wq1sb[:, dc, hc * 128:(hc + 1) * 128],
                            rhs=xts[dc][:, nt * 512:(nt + 1) * 512],
                            start=(dc == 0),
                            stop=(dc == 7),
                        )
                    nc.scalar.activation(
                        out=qt8[:, hc, nt * 512:(nt + 1) * 512], in_=ps,
                        func=Ident, bias=cpack[:, 2 + hc:2 + hc + 1], scale=SCQ,
                    )

            # ---- constants for transposes / reduce stationaries ----
            ident = consts.tile([16, 16], f32)
            make_identity(nc, ident)
            ident8 = consts.tile([8, 8], fp8)
            nc.vector.tensor_copy(out=ident8, in_=ident[0:8, 0:8])
            # z columns in fp8, one per (rank, chunk-in-rank): zcat8[:, r*8+c]
            zcat8 = consts.tile([128, 64], fp8)
            # all 32 pair stationaries [128, pair, 2, 128]: cols 0:64 the two
            # z chunks replicated (broadcast DVE copies per (split, cp)),
            # cols 64:128 ones.  Pair order: split 0 pairs (r), then split 1
            # pairs (r, cp); pair chunks g = r*8 + split_base + 2*cp + j.
            # 64-col stationary (not 128): halves its LDWEIGHTS cost; out
            # rows 0..31 = num, 32..63 = den (still 32-partition aligned)
            zrall = consts.tile([128, 32, 2, 64], fp8)
            for j in range(2):
                nc.vector.memset(zrall[:, :, j, 32:64], 1.0)
            zc3 = zcat8.rearrange("p (r f) -> p r f", f=8)

            # ---- main loop ----
            kt8 = work.tile([128, 2, N], fp8)
            cc3 = [
                cc_out[t].rearrange("(r q) c -> r q c", q=H + 1)
                for t in range(len(SPLITS))
            ]
            cc3q = [
                cc_out[t].rearrange("(r q) c -> q r c", q=H + 1)
                for t in range(len(SPLITS))
            ]
            kt8r = kt8.rearrange("p i (r n) -> p i r n", r=NC)
            zrows8 = [
                work.tile([8, e - s, 1], fp8, name=f"zrows8_{t}")
                for t, (s, e) in enumerate(SPLITS)
            ]

            psred = [
                redp.tile([64, 512], f32, tag=f"red{t}", name=f"psred{t}")
                for t in range(2)
            ]

            # pair bookkeeping: split 0 -> pis 0..7 (cp=0), split 1 ->
            # pis 8..31 as (r, cp in 0..2)
            pair_base = [0, NC * CQS[0] // 2]

            def emit_load_dmas(t):
                cs, ce = SPLITS[t]
                # kt columns: rank r split t -> global cols r*S + cs.  The
                # big split's DMAs are split by rank pair so early pairs can
                # score while later ranks' columns still transfer; the small
                # split loads in one piece (dma_start issue cost dominates)
                nc.sync.dma_start(out=zrows8[t][:, :, 0], in_=cc3[t][:, H, :])
                # one DMA per rank (both hc halves), fanned across DGE
                # queues: each dma_start costs ~650ns of serial issue, and
                # pair 0 needs only rank 0's 64KB.  Split 0 uses sync+vector
                # (vector is idle until the first exp); split 1 uses
                # sync+gpsimd (gpsimd's queue is blocked by AG1 until then).
                for r in range(NC):
                    if t == 0:
                        eng = nc.sync if r < 4 else nc.scalar
                    else:
                        eng = nc.sync if r % 2 == 0 else nc.gpsimd
                    eng.dma_start(
                        out=kt8r[:, :, r, cs:ce],
                        in_=cc3[t][r, 0:H, :].rearrange("(i p) c -> p i c", p=128),
                    )

            def emit_load_pe(t):
                cs, ce = SPLITS[t]
                cq = CQS[t]
                for f in range(cq):
                    # fp8 transpose mode requires output element step of 2
                    pzt = stp.tile([128, 8, 2], fp8, tag="st", name="pzt")
                    nc.tensor.transpose(
                        out=pzt[:, :, 0],
                        in_=zrows8[t][:, f * 128:(f + 1) * 128, 0],
                        identity=ident8,
                    )
                    nc.vector.tensor_copy(
                        out=zc3[:, :, cs // 128 + f], in_=pzt[:, :, 0]
                    )
                # replicate this split's z columns into its pair stationaries
                for cp in range(cq // 2):
                    c0 = cs // 128 + 2 * cp
                    src = zc3[:, :, c0:c0 + 2]
                    dst = zrall[:, pair_base[t]:pair_base[t] + NC * cq // 2]
                    dst = dst.rearrange("p (r cp) j c -> p r cp j c", cp=cq // 2)
                    nc.vector.tensor_copy(
                        out=dst[:, :, cp, :, 0:32],
                        in_=src[:, :, :, None].broadcast_to([128, 8, 2, 32]),
                    )

            sched = []
            for t, (cs, ce) in enumerate(SPLITS):
                for r in range(NC):
                    for cp in range(CQS[t] // 2):
                        chunks = []
                        for j in range(2):
                            g = r * 8 + cs // 128 + 2 * cp + j
                            chunks.append(kt8[:, :, g * 128:(g + 1) * 128])
                        sched.append(chunks)

            e2s = {}

            def emit_score(pi):
                chunks = sched[pi]
                e2 = expp.tile([128, 2, S], fp8, tag="e2", name="e2")
                for j, ktap in enumerate(chunks):
                    st = stp.tile([128, 1024], f32, tag="st", name="st")
                    for nt in range(2):
                        nc.tensor.matmul(
                            st[:, nt * 512:(nt + 1) * 512],
                            lhsT=ktap,
                            rhs=qt8[:, :, nt * 512:(nt + 1) * 512],
                            perf_mode=DR,
                        )
                    # exp split across ACT and DVE by measured rates
                    # (1.34 vs 1.47 us/tile): j==1 tiles go to the DVE
                    # (Schraudolph int8 bitcast) except pairs 14/30 (not 31:
                    # the final pair's tiles must finish on both engines in
                    # parallel so the last reduce isn't serialized)
                    if j == 1 and pi % 16 != 14:
                        nc.vector.tensor_scalar(
                            out=e2[:, j, :].bitcast(i8), in0=st,
                            scalar1=A8, scalar2=B8, op0=Mul, op1=Add,
                        )
                    else:
                        nc.scalar.activation(
                            out=e2[:, j, :], in_=st, func=Exp, scale=SC_EXP
                        )
                e2s[pi] = e2

            def emit_reduce(pi):
                e2 = e2s.pop(pi)
                zr = zrall[:, pi, :, :]
                for nt in range(2):
                    nc.tensor.matmul(
                        psred[nt],
                        lhsT=zr,
                        rhs=e2[:, :, nt * 512:(nt + 1) * 512],
                        perf_mode=DR,
                        start=(pi == 0),
                        stop=(pi == NPAIR - 1),
                    )

            for pi in range(NPAIR + LAG):
                if pi == 0:
                    emit_load_dmas(0)
                    emit_load_pe(0)
                if pi == 5:
                    emit_load_dmas(1)
                if pi == 8:
                    emit_load_pe(1)
                if pi < NPAIR:
                    emit_score(pi)
                if pi >= LAG:
                    emit_reduce(pi - LAG)

            # ---- epilogue: out = resid + num/den ----
            # psred rows: 0..31 num copies, 32..63 den copies
            dsb = consts.tile([33, S], f32)
            for nt in range(2):
                nc.vector.tensor_copy(
                    out=dsb[32:33, nt * 512:(nt + 1) * 512], in_=psred[nt][32:33, :]
                )
            dall = consts.tile([1, S], f32)
            nc.sync.dma_start(out=dall, in_=dsb[32:33, :])
            rden = consts.tile([1, S], f32)
            nc.vector.reciprocal_approx_fast(out=rden, in_=dall)
            m = consts.tile([1, S], f32)
            for nt in range(2):
                nc.vector.tensor_mul(
                    m[:, nt * 512:(nt + 1) * 512],
                    psred[nt][0:1, :],
                    rden[:, nt * 512:(nt + 1) * 512],
                )
            outsb = consts.tile([1, S], f32)
            nc.vector.tensor_add(outsb, m, residsb)
            nc.sync.dma_start(out=out_d[:], in_=outsb)

    nc.compile()
    return nc


def _get_program():
    if "nc" not in _cache:
        _cache["nc"] = _build_program()
    return _cache["nc"]


def kernel(x, lin1_w, lin1_b, q_w, q_b, k_w, k_b, v_w, v_b, lin2_w, lin2_b):
    from concourse.bass_utils import run_bass_kernel_spmd

    x = np.asarray(x, dtype=np.float32)
    lin1_w = np.asarray(lin1_w, dtype=np.float32)
    lin1_b = np.asarray(lin1_b, dtype=np.float32)
    q_w = np.asarray(q_w, dtype=np.float32)
    q_b = np.asarray(q_b, dtype=np.float32)
    k_w = np.asarray(k_w, dtype=np.float32)
    k_b = np.asarray(k_b, dtype=np.float32)
    v_w = np.asarray(v_w, dtype=np.float32)
    v_b = np.asarray(v_b, dtype=np.float32)
    lin2_w = np.asarray(lin2_w, dtype=np.float32)
    lin2_b = np.asarray(lin2_b, dtype=np.float32)

    nc = _get_program()

    wk1 = (k_w.astype(np.float64) @ lin1_w.astype(np.float64)).astype(np.float32)
    bk1 = (k_w.astype(np.float64) @ lin1_b.astype(np.float64)).astype(np.float32) + k_b
    wq1 = (q_w.astype(np.float64) @ lin1_w.astype(np.float64)).astype(np.float32)
    bq1 = (q_w.astype(np.float64) @ lin1_b.astype(np.float64)).astype(np.float32) + q_b
    w2 = lin2_w[0]                                  # [H]
    wv2 = (v_w.T.astype(np.float64) @ w2.astype(np.float64)).astype(np.float32)
    zw = (lin1_w.T.astype(np.float64) @ wv2.astype(np.float64)).astype(np.float32)
    zc0 = np.float32(wv2.astype(np.float64) @ lin1_b.astype(np.float64))
    wr = (lin1_w.T.astype(np.float64) @ w2.astype(np.float64)).astype(np.float32)
    cr = np.float32(lin1_b @ w2 + v_b @ w2 + lin2_b[0])

    cpk = np.zeros((128, 16), dtype=np.float32)
    cpk[:, 0:2] = (bk1 * SCQ).reshape(2, 128).T
    cpk[:, 2:4] = (bq1 * SCQ).reshape(2, 128).T
    cpk[0, 4] = zc0
    cpk[1, 4] = cr
    zwp = np.zeros((128, 16), dtype=np.float32)
    zwp[:, 0::2] = zw.reshape(8, 128).T
    zwp[:, 1::2] = wr.reshape(8, 128).T

    wk1T = np.ascontiguousarray(wk1.T)              # [D, H]
    wq1T = np.ascontiguousarray(wq1.T)              # [D, H]

    in_maps = []
    for i in range(NC):
        in_maps.append({
            "xT": np.ascontiguousarray(x[i * S:(i + 1) * S, :].T),
            "wk1T": wk1T, "wq1T": wq1T,
            "cpk": cpk, "zwp": zwp,
        })

    res = run_bass_kernel_spmd(nc, in_maps, core_ids=list(range(NC)))
    out = np.concatenate([res.results[i]["out"].reshape(S) for i in range(NC)])
    return out.astype(np.float32)
